# revision 70
# baseline (speedup 1.0000x reference)
"""AConnect (nn_AConnect_82368882803074) Trainium2 kernel.

Reference computation:
    memW[b]    = W * Werr_bank[idx[b]]             [B, D_in, D_out]
    membias[b] = bias * Berr_bank[idx[b]]          [B, 1, D_out]
    Z[b]       = X[b] @ memW[b] + membias[b]       [B, D_out]

Strategy: data-parallel over the batch across 8 NeuronCores, with
duplicate-bank dedup. The host groups samples by bank index and packs the
banks onto cores ("slots"); each slot loads its bank matrix once and carries
up to M=4 samples as extra matmul columns. The host only moves data (gather,
transpose, zero-padding, output permutation); all arithmetic (W ⊙ E,
X @ (W ⊙ E), bias ⊙ Berr and the final add) runs on device.

Per core the device kernel streams K gathered 1 MB bank matrices from HBM,
casting f32->bf16 inside the (SWDGE) DMA, multiplies by W on VectorE in bf16
(2x mode), and contracts with the slot's 4 X-columns on TensorE (4 k-chunk
matmuls accumulating into a [4, 512] PSUM tile). VectorE then adds the
bias term while draining PSUM into a small output tile, which the (otherwise
idle) scalar-ring DMA writes straight to the slot's 4 output rows in DRAM.
Dummy matmuls on resident tiles keep the PE's HAM activity monitor busy so
real matmuls run at 2.4 GHz instead of 1.2.
"""

import numpy as np

B, D_IN, D_OUT, N_BANK, N_CORES = 256, 512, 512, 1000, 8
P = 128  # partitions
C = D_IN // P  # 4 k-chunks
M = 4  # samples per bank slot (max observed bank multiplicity is 3)

_CACHE = {}
last_exec_time_ns = None


def _build_nc(K):
    """Device graph for K bank-slots per core."""
    import concourse.mybir as mybir
    import concourse.tile as tile
    from concourse import bacc

    f32 = mybir.dt.float32
    bf16 = mybir.dt.bfloat16
    nc = bacc.Bacc()

    R = K * M  # output rows, slot-major: row t*M + j = slot t, column j
    eg = nc.dram_tensor("eg", [K, P, C * D_OUT], f32, kind="ExternalInput")
    wt = nc.dram_tensor("wt", [P, C * D_OUT], f32, kind="ExternalInput")
    xtt = nc.dram_tensor("xtt", [P, C * R], f32, kind="ExternalInput")
    bb = nc.dram_tensor("bb", [K, D_OUT], f32, kind="ExternalInput")
    beg = nc.dram_tensor("beg", [K, D_OUT], f32, kind="ExternalInput")
    out = nc.dram_tensor("out", [R, D_OUT], f32, kind="ExternalOutput")

    with tile.TileContext(nc) as tc:
        with (
            tc.tile_pool(name="const", bufs=1) as constp,
            tc.tile_pool(name="ep", bufs=12) as ep,
            tc.tile_pool(name="wep", bufs=9) as wep,
            tc.tile_pool(name="ps", bufs=7, space="PSUM") as psp,
            tc.tile_pool(name="scr", bufs=2) as scr,
            tc.tile_pool(name="outp", bufs=8) as outp,
        ):
            w_t = constp.tile([P, C * D_OUT], f32)
            nc.sync.dma_start(w_t[:], wt[:])
            x_t = constp.tile([P, C * R], f32)
            nc.sync.dma_start(x_t[:], xtt[:])

            # membias = bias * Berr[bank] — one row per bank slot, in bf16
            # (it joins the bf16 PE accumulation below)
            bias_k = scr.tile([K, D_OUT], f32, name="bias_k", tag="bq")
            nc.sync.dma_start(bias_k[:], bb[:])
            berr_k = scr.tile([K, D_OUT], f32, name="berr_k", tag="eq")
            nc.sync.dma_start(berr_k[:], beg[:])
            mbk = constp.tile([K, D_OUT], bf16, name="mbk")
            nc.vector.tensor_mul(mbk[:], bias_k[:], berr_k[:])

            # bf16 copies of the resident matmul operands
            w_b = constp.tile([P, C * D_OUT], bf16)
            nc.vector.tensor_copy(w_b[:], w_t[:])
            x_b = constp.tile([P, C * R], bf16)
            nc.vector.tensor_copy(x_b[:], x_t[:])

            # Dummy matmuls on resident tiles keep the PE's HAM activity
            # monitor busy so real matmuls run at 2.4 GHz instead of 1.2.
            warm = psp.tile([M, D_OUT], f32, name="warm", bufs=1)

            def warm_mm(n=D_OUT):
                nc.tensor.matmul(
                    warm[:, 0:n], x_b[:, 0:M], w_b[:, 0:n], start=True, stop=True
                )

            for _ in range(16):
                warm_mm()

            # membias rows relocated to one partition-0 strip in a single
            # reshape DMA (the k=1 bias matmul below needs its rhs at
            # partition 0; src iterates (t, n), dst (0, t*512+n))
            mbrow = constp.tile([1, K * D_OUT], bf16)
            nc.scalar.dma_start(mbrow[:], mbk[:])
            ones_b = constp.tile([1, M], bf16)
            nc.any.memset(ones_b[:], 1.0)

            for t in range(K):
                # f32 -> bf16 cast happens inside the (SWDGE) DMA
                eb = ep.tile([P, C * D_OUT], bf16)
                nc.gpsimd.dma_start(eb[:], eg[t])
                we = wep.tile([P, C * D_OUT], bf16)
                nc.vector.tensor_mul(we[:], eb[:], w_b[:])
                ps = psp.tile([M, D_OUT], f32)
                for c in range(C):
                    nc.tensor.matmul(
                        ps[:],
                        x_b[:, (c * K + t) * M : (c * K + t) * M + M],
                        we[:, c * D_OUT : (c + 1) * D_OUT],
                        start=(c == 0),
                        stop=False,
                    )
                # bias joins the PSUM accumulation: ones[1,M]^T @ membias[1,N]
                # broadcasts the bank's membias row onto all M output rows
                # (and doubles as the PE warm-keeper)
                nc.tensor.matmul(
                    ps[:],
                    ones_b[:],
                    mbrow[0:1, t * D_OUT : (t + 1) * D_OUT],
                    start=False,
                    stop=True,
                )
                # drain PSUM on the otherwise idle ScalarE; slot pairs share
                # one staging tile and one store DMA
                u = t % 2
                if u == 0:
                    osb = outp.tile([M, 2 * D_OUT], f32, name="osb", tag="osb")
                nc.scalar.copy(osb[0:M, u * D_OUT : (u + 1) * D_OUT], ps[:])
                if u == 1 or t == K - 1:
                    t0 = t - u
                    nrows = (u + 1) * M
                    nc.scalar.dma_start(
                        out[t0 * M : t0 * M + nrows, :].rearrange(
                            "(v j) n -> j v n", j=M
                        ),
                        osb[0:M, 0 : (u + 1) * D_OUT].rearrange(
                            "j (v n) -> j v n", n=D_OUT
                        ),
                    )

    nc.compile()
    return nc


def _pack(idx):
    """Group samples by bank, pack banks onto cores.

    Returns (K, plan) where plan[c] is a list of (bank, [samples]) slots,
    each slot carrying at most M samples of one bank.
    """
    from collections import defaultdict

    groups = defaultdict(list)
    for s, b in enumerate(idx):
        groups[int(b)].append(s)
    # one slot per <=M samples of a bank
    slots = []
    for b, ss in groups.items():
        for i in range(0, len(ss), M):
            slots.append((b, ss[i : i + M]))
    slots.sort(key=lambda x: -len(x[1]))
    plan = [[] for _ in range(N_CORES)]
    for b, ss in slots:
        c = min(range(N_CORES), key=lambda c: len(plan[c]))
        plan[c].append((b, ss))
    K = max(len(p) for p in plan)
    return K, plan


def _install_trace_shim():
    """Register the axon NTFF profile hook bass_utils expects (the agent
    image lacks antenv.axon_hooks; the C ABI is in libaxon_pjrt.so)."""
    import contextlib
    import ctypes
    import sys
    import types

    if "antenv.axon_hooks" in sys.modules:
        return
    lib = ctypes.CDLL("/opt/axon/libaxon_pjrt.so")
    if not hasattr(lib, "axon_start_nrt_profile"):
        hook = None
    else:
        lib.axon_start_nrt_profile.argtypes = [
            ctypes.POINTER(ctypes.c_int64),
            ctypes.c_size_t,
        ]
        lib.axon_start_nrt_profile.restype = ctypes.c_int64
        lib.axon_stop_nrt_profile.argtypes = [ctypes.c_char_p]
        lib.axon_stop_nrt_profile.restype = ctypes.c_int64

        @contextlib.contextmanager
        def hook(output_dir, device_ids):
            import jax

            jax.devices()
            if device_ids:
                ids = (ctypes.c_int64 * len(device_ids))(*device_ids)
                rc = lib.axon_start_nrt_profile(ids, len(device_ids))
            else:
                rc = lib.axon_start_nrt_profile(None, 0)
            if rc != 0:
                raise RuntimeError(f"axon_start_nrt_profile rc={rc}")
            try:
                yield
            finally:
                n = lib.axon_stop_nrt_profile(str(output_dir).encode())
                print(f"ntff profile: {n} file(s) -> {output_dir}", file=sys.stderr)

    mod = types.ModuleType("antenv.axon_hooks")
    mod.get_axon_ntff_profile_hook = lambda: hook
    mod.set_axon_ntff_profile_hook = lambda h: None
    sys.modules["antenv.axon_hooks"] = mod


def kernel(X, W, bias, Werr_bank, Berr_bank, idx):
    global last_exec_time_ns
    import os

    from concourse.bass_utils import run_bass_kernel_spmd

    X = np.asarray(X, dtype=np.float32)
    W = np.asarray(W, dtype=np.float32)
    bias = np.asarray(bias, dtype=np.float32)
    Werr_bank = np.asarray(Werr_bank, dtype=np.float32)
    Berr_bank = np.asarray(Berr_bank, dtype=np.float32)
    idx = np.asarray(idx, dtype=np.int32)

    K, plan = _pack(idx)
    if ("nc", K) not in _CACHE:
        _CACHE[("nc", K)] = _build_nc(K)
    nc = _CACHE[("nc", K)]
    R = K * M

    # Host-side sharding / layout (pure data movement).
    wt = np.ascontiguousarray(
        W.reshape(C, P, D_OUT).transpose(1, 0, 2).reshape(P, C * D_OUT)
    )
    bb = np.ascontiguousarray(np.broadcast_to(bias.reshape(1, D_OUT), (K, D_OUT)))

    in_maps = []
    row_of_sample = np.full(B, -1, dtype=np.int64)  # (core, row) flattened
    for c_id in range(N_CORES):
        slots = plan[c_id]
        banks = [b for b, _ in slots] + [0] * (K - len(slots))
        eg = Werr_bank[banks]  # [K, D_in, D_out]
        eg = np.ascontiguousarray(
            eg.reshape(K, C, P, D_OUT).transpose(0, 2, 1, 3).reshape(K, P, C * D_OUT)
        )
        # X columns and output rows in slot-major order: row t*M + j
        xs = np.zeros((R, D_IN), dtype=np.float32)
        beg = np.ascontiguousarray(Berr_bank[banks, 0, :])  # [K, D_out]
        for t, (b, ss) in enumerate(slots):
            for j, s in enumerate(ss):
                xs[t * M + j] = X[s]
                row_of_sample[s] = c_id * R + t * M + j
        xtt = np.ascontiguousarray(
            xs.T.reshape(C, P, R).transpose(1, 0, 2).reshape(P, C * R)
        )
        in_maps.append({"eg": eg, "wt": wt, "xtt": xtt, "bb": bb, "beg": beg})
    assert (row_of_sample >= 0).all()

    trace = os.environ.get("BASS_KERNEL_TRACE") == "1"
    if trace:
        _install_trace_shim()
    res = run_bass_kernel_spmd(
        nc,
        in_maps,
        core_ids=list(range(N_CORES)),
        trace=trace,
        trace_cores=(
            list(range(N_CORES))
            if os.environ.get("BASS_KERNEL_TRACE_ALL") == "1"
            else [0]
        )
        if trace
        else None,
    )
    last_exec_time_ns = res.exec_time_ns
    allrows = np.concatenate([r["out"] for r in res.results], axis=0)  # [8*R, 512]
    return np.ascontiguousarray(allrows[row_of_sample])


# revision 73
# speedup vs baseline: 1.1603x; 1.1603x over previous
"""AConnect (nn_AConnect_82368882803074) Trainium2 kernel.

Reference computation:
    memW[b]    = W * Werr_bank[idx[b]]             [B, D_in, D_out]
    membias[b] = bias * Berr_bank[idx[b]]          [B, 1, D_out]
    Z[b]       = X[b] @ memW[b] + membias[b]       [B, D_out]

Strategy: data-parallel over the batch across 8 NeuronCores, with
duplicate-bank dedup. The host groups samples by bank index and packs the
banks onto cores ("slots"); each slot loads its bank matrix once and carries
up to M=4 samples as extra matmul columns. The host only moves data (gather,
transpose, zero-padding, output permutation); all arithmetic (W ⊙ E,
X @ (W ⊙ E), bias ⊙ Berr and the final add) runs on device.

Per core the device kernel streams K gathered 1 MB bank matrices from HBM,
casting f32->bf16 inside the (SWDGE) DMA, multiplies by W on VectorE in bf16
(2x mode), and contracts with the slot's 4 X-columns on TensorE (4 k-chunk
matmuls accumulating into a [4, 512] PSUM tile). VectorE then adds the
bias term while draining PSUM into a small output tile, which the (otherwise
idle) scalar-ring DMA writes straight to the slot's 4 output rows in DRAM.
Dummy matmuls on resident tiles keep the PE's HAM activity monitor busy so
real matmuls run at 2.4 GHz instead of 1.2.
"""

import numpy as np

B, D_IN, D_OUT, N_BANK, N_CORES = 256, 512, 512, 1000, 8
P = 128  # partitions
C = D_IN // P  # 4 k-chunks
M = 4  # samples per bank slot (max observed bank multiplicity is 3)

_CACHE = {}
last_exec_time_ns = None


def _build_nc(K):
    """Device graph for K bank-slots per core."""
    import concourse.mybir as mybir
    import concourse.tile as tile
    from concourse import bacc

    f32 = mybir.dt.float32
    bf16 = mybir.dt.bfloat16
    nc = bacc.Bacc()

    R = K * M  # output rows, slot-major: row t*M + j = slot t, column j
    eg = nc.dram_tensor("eg", [K, P, C * D_OUT], f32, kind="ExternalInput")
    wt = nc.dram_tensor("wt", [P, C * D_OUT], f32, kind="ExternalInput")
    xtt = nc.dram_tensor("xtt", [P, C * R], f32, kind="ExternalInput")
    bb = nc.dram_tensor("bb", [K, D_OUT], f32, kind="ExternalInput")
    beg = nc.dram_tensor("beg", [K, D_OUT], f32, kind="ExternalInput")
    out = nc.dram_tensor("out", [R, D_OUT], f32, kind="ExternalOutput")

    with tile.TileContext(nc) as tc:
        with (
            tc.tile_pool(name="const", bufs=1) as constp,
            tc.tile_pool(name="ep", bufs=12) as ep,
            tc.tile_pool(name="wep", bufs=9) as wep,
            tc.tile_pool(name="ps", bufs=7, space="PSUM") as psp,
            tc.tile_pool(name="scr", bufs=2) as scr,
            tc.tile_pool(name="outp", bufs=8) as outp,
        ):
            w_t = constp.tile([P, C * D_OUT], f32)
            nc.scalar.dma_start(w_t[:], wt[:])
            x_t = constp.tile([P, C * R], f32)
            nc.scalar.dma_start(x_t[:], xtt[:])

            # membias = bias * Berr[bank] — one row per bank slot, in bf16
            # (it joins the bf16 PE accumulation below)
            bias_k = scr.tile([K, D_OUT], f32, name="bias_k", tag="bq")
            nc.scalar.dma_start(bias_k[:], bb[:])
            berr_k = scr.tile([K, D_OUT], f32, name="berr_k", tag="eq")
            nc.scalar.dma_start(berr_k[:], beg[:])
            mbk = constp.tile([K, D_OUT], bf16, name="mbk")
            nc.vector.tensor_mul(mbk[:], bias_k[:], berr_k[:])

            # bf16 copies of the resident matmul operands
            w_b = constp.tile([P, C * D_OUT], bf16)
            nc.vector.tensor_copy(w_b[:], w_t[:])
            x_b = constp.tile([P, C * R], bf16)
            nc.vector.tensor_copy(x_b[:], x_t[:])

            # Dummy matmuls on resident tiles keep the PE's HAM activity
            # monitor busy so real matmuls run at 2.4 GHz instead of 1.2.
            warm = psp.tile([M, D_OUT], f32, name="warm", bufs=1)

            def warm_mm(n=D_OUT):
                nc.tensor.matmul(
                    warm[:, 0:n], x_b[:, 0:M], w_b[:, 0:n], start=True, stop=True
                )

            for _ in range(16):
                warm_mm()

            # membias rows relocated to one partition-0 strip in a single
            # reshape DMA (the k=1 bias matmul below needs its rhs at
            # partition 0; src iterates (t, n), dst (0, t*512+n))
            mbrow = constp.tile([1, K * D_OUT], bf16)
            nc.scalar.dma_start(mbrow[:], mbk[:])
            ones_b = constp.tile([1, M], bf16)
            nc.any.memset(ones_b[:], 1.0)

            # The first few banks load on the HWDGE sync ring (which needs no
            # engine library and starts ~5µs before SWDGE can) as f32, cast
            # by the then-idle VectorE; the SWDGE cast-DMA takes over after.
            NPRE = 3
            for t in range(K):
                eb = ep.tile([P, C * D_OUT], bf16)
                if t < NPRE:
                    ef = scr.tile([P, C * D_OUT], f32, name=f"ef{t}", tag="ef")
                    nc.sync.dma_start(ef[:], eg[t])
                    nc.vector.tensor_copy(eb[:], ef[:])
                else:
                    # f32 -> bf16 cast happens inside the (SWDGE) DMA
                    nc.gpsimd.dma_start(eb[:], eg[t])
                we = wep.tile([P, C * D_OUT], bf16)
                nc.vector.tensor_mul(we[:], eb[:], w_b[:])
                ps = psp.tile([M, D_OUT], f32)
                for c in range(C):
                    nc.tensor.matmul(
                        ps[:],
                        x_b[:, (c * K + t) * M : (c * K + t) * M + M],
                        we[:, c * D_OUT : (c + 1) * D_OUT],
                        start=(c == 0),
                        stop=False,
                    )
                # bias joins the PSUM accumulation: ones[1,M]^T @ membias[1,N]
                # broadcasts the bank's membias row onto all M output rows
                # (and doubles as the PE warm-keeper)
                nc.tensor.matmul(
                    ps[:],
                    ones_b[:],
                    mbrow[0:1, t * D_OUT : (t + 1) * D_OUT],
                    start=False,
                    stop=True,
                )
                # drain PSUM on the otherwise idle ScalarE; slot pairs share
                # one staging tile and one store DMA
                u = t % 2
                if u == 0:
                    osb = outp.tile([M, 2 * D_OUT], f32, name="osb", tag="osb")
                nc.scalar.copy(osb[0:M, u * D_OUT : (u + 1) * D_OUT], ps[:])
                if u == 1 or t == K - 1:
                    t0 = t - u
                    nrows = (u + 1) * M
                    nc.scalar.dma_start(
                        out[t0 * M : t0 * M + nrows, :].rearrange(
                            "(v j) n -> j v n", j=M
                        ),
                        osb[0:M, 0 : (u + 1) * D_OUT].rearrange(
                            "j (v n) -> j v n", n=D_OUT
                        ),
                    )

    nc.compile()
    return nc


def _pack(idx):
    """Group samples by bank, pack banks onto cores.

    Returns (K, plan) where plan[c] is a list of (bank, [samples]) slots,
    each slot carrying at most M samples of one bank.
    """
    from collections import defaultdict

    groups = defaultdict(list)
    for s, b in enumerate(idx):
        groups[int(b)].append(s)
    # one slot per <=M samples of a bank
    slots = []
    for b, ss in groups.items():
        for i in range(0, len(ss), M):
            slots.append((b, ss[i : i + M]))
    slots.sort(key=lambda x: -len(x[1]))
    plan = [[] for _ in range(N_CORES)]
    for b, ss in slots:
        c = min(range(N_CORES), key=lambda c: len(plan[c]))
        plan[c].append((b, ss))
    K = max(len(p) for p in plan)
    return K, plan


def _install_trace_shim():
    """Register the axon NTFF profile hook bass_utils expects (the agent
    image lacks antenv.axon_hooks; the C ABI is in libaxon_pjrt.so)."""
    import contextlib
    import ctypes
    import sys
    import types

    if "antenv.axon_hooks" in sys.modules:
        return
    lib = ctypes.CDLL("/opt/axon/libaxon_pjrt.so")
    if not hasattr(lib, "axon_start_nrt_profile"):
        hook = None
    else:
        lib.axon_start_nrt_profile.argtypes = [
            ctypes.POINTER(ctypes.c_int64),
            ctypes.c_size_t,
        ]
        lib.axon_start_nrt_profile.restype = ctypes.c_int64
        lib.axon_stop_nrt_profile.argtypes = [ctypes.c_char_p]
        lib.axon_stop_nrt_profile.restype = ctypes.c_int64

        @contextlib.contextmanager
        def hook(output_dir, device_ids):
            import jax

            jax.devices()
            if device_ids:
                ids = (ctypes.c_int64 * len(device_ids))(*device_ids)
                rc = lib.axon_start_nrt_profile(ids, len(device_ids))
            else:
                rc = lib.axon_start_nrt_profile(None, 0)
            if rc != 0:
                raise RuntimeError(f"axon_start_nrt_profile rc={rc}")
            try:
                yield
            finally:
                n = lib.axon_stop_nrt_profile(str(output_dir).encode())
                print(f"ntff profile: {n} file(s) -> {output_dir}", file=sys.stderr)

    mod = types.ModuleType("antenv.axon_hooks")
    mod.get_axon_ntff_profile_hook = lambda: hook
    mod.set_axon_ntff_profile_hook = lambda h: None
    sys.modules["antenv.axon_hooks"] = mod


def kernel(X, W, bias, Werr_bank, Berr_bank, idx):
    global last_exec_time_ns
    import os

    from concourse.bass_utils import run_bass_kernel_spmd

    X = np.asarray(X, dtype=np.float32)
    W = np.asarray(W, dtype=np.float32)
    bias = np.asarray(bias, dtype=np.float32)
    Werr_bank = np.asarray(Werr_bank, dtype=np.float32)
    Berr_bank = np.asarray(Berr_bank, dtype=np.float32)
    idx = np.asarray(idx, dtype=np.int32)

    K, plan = _pack(idx)
    if ("nc", K) not in _CACHE:
        _CACHE[("nc", K)] = _build_nc(K)
    nc = _CACHE[("nc", K)]
    R = K * M

    # Host-side sharding / layout (pure data movement).
    wt = np.ascontiguousarray(
        W.reshape(C, P, D_OUT).transpose(1, 0, 2).reshape(P, C * D_OUT)
    )
    bb = np.ascontiguousarray(np.broadcast_to(bias.reshape(1, D_OUT), (K, D_OUT)))

    in_maps = []
    row_of_sample = np.full(B, -1, dtype=np.int64)  # (core, row) flattened
    for c_id in range(N_CORES):
        slots = plan[c_id]
        banks = [b for b, _ in slots] + [0] * (K - len(slots))
        eg = Werr_bank[banks]  # [K, D_in, D_out]
        eg = np.ascontiguousarray(
            eg.reshape(K, C, P, D_OUT).transpose(0, 2, 1, 3).reshape(K, P, C * D_OUT)
        )
        # X columns and output rows in slot-major order: row t*M + j
        xs = np.zeros((R, D_IN), dtype=np.float32)
        beg = np.ascontiguousarray(Berr_bank[banks, 0, :])  # [K, D_out]
        for t, (b, ss) in enumerate(slots):
            for j, s in enumerate(ss):
                xs[t * M + j] = X[s]
                row_of_sample[s] = c_id * R + t * M + j
        xtt = np.ascontiguousarray(
            xs.T.reshape(C, P, R).transpose(1, 0, 2).reshape(P, C * R)
        )
        in_maps.append({"eg": eg, "wt": wt, "xtt": xtt, "bb": bb, "beg": beg})
    assert (row_of_sample >= 0).all()

    trace = os.environ.get("BASS_KERNEL_TRACE") == "1"
    if trace:
        _install_trace_shim()
    res = run_bass_kernel_spmd(
        nc,
        in_maps,
        core_ids=list(range(N_CORES)),
        trace=trace,
        trace_cores=(
            list(range(N_CORES))
            if os.environ.get("BASS_KERNEL_TRACE_ALL") == "1"
            else [0]
        )
        if trace
        else None,
    )
    last_exec_time_ns = res.exec_time_ns
    allrows = np.concatenate([r["out"] for r in res.results], axis=0)  # [8*R, 512]
    return np.ascontiguousarray(allrows[row_of_sample])
